# revision 15
# baseline (speedup 1.0000x reference)
"""Distributed KNN (k-nearest-neighbor classify) on 8 Trainium2 NeuronCores.

Sharding: X_train/y_train split along num_train across 8 cores. Candidates
are globally sorted by ||t||^2 and dealt round-robin to cores; the extreme
norm tails go to 1:1 "leftover" slots and each middle 4096-candidate block is
interleaved on the host so every folded slot's 32-member preimage is 32
CONSECUTIVE-sorted candidates (a tight norm stratum).

Per core, per 128-test group g (8 groups):
  TensorE (fp16): raw dots d[t,n] = X[t].Xtr[n] into PSUM fp32: six 4-bank
    [128,2048] tiles + one 212-wide leftover. No per-candidate bias matmul.
  Egress + max-tree: ScalarE casts even tiles (and, for CONVERT pairs, both)
    to fp16 SBUF; DVE tensor_max folds odd PSUM tiles against them (L1), then
    fp16 TT-max folds at 2x mode (multi-block 3D APs, one op per level)
    reduce 6144 pair-maxima -> 384 slots + 212 leftover slots = 596.
  Slot bias: V += sbias (broadcast [128,596] built once per core by K=1
    matmuls from a host row of -max(||t||^2 over the slot)/2; exact for
    leftover slots). Values are then upper bounds of the slot's best adjusted
    score up to fp16 noise, so no true-top slot is ever demoted.
  DVE MAX8 + FIND_INDEX8 give top-8 (value, slot) per test per core.

Host: merges 8 cores x 8 slots per test, rescores the top-8 sets exactly in
float64, then adaptively expands every further set whose upper bound clears
the provisional kth value (branch-and-bound), takes exact top-k (value desc,
index asc = lax.top_k semantics), labels, majority vote (ties -> smallest).
"""
import numpy as np
from contextlib import ExitStack

# Problem geometry (hardcoded per contract).
D = 128          # feature dim = contraction dim
T = 1024         # num test points
N_TRAIN = 100000
N_CORES = 8
NS = N_TRAIN // N_CORES   # 12500 train points per core
NG = T // 128             # 8 test groups of 128 (PSUM partition dim)
BANK = 512                # fp32 elems per PSUM bank (matmul max N)
PTW = 2048                # psum tile width (4 banks)
NPAIR = 3                 # pairs of psum tiles per group (6 tiles = 12288)
NMID = NPAIR * 2 * PTW    # 12288 mid candidates in the folded region
LEFT = NS - NMID          # 212 leftover candidates (1:1 slots)
FB = 128                  # folded slots per 4096-block (preimage 32)
NSLOT = NPAIR * FB + LEFT  # 596 selection slots per (test, core)
NKEEP = 8                 # top slots kept per (test, core)
NUM_CLASSES = 10
TAILL = 100               # lowest-norm ranks routed to 1:1 leftover slots
CONVERT = 2               # pairs where ScalarE casts both tiles per group

_CACHE = {}


def _pos_to_sorted():
    """device position p (0..NS) -> within-core sorted rank j.

    Middle positions fill the folded region so each final slot's 32-member
    preimage is 32 consecutive sorted ranks starting at TAILL; the extreme
    norm tails (lowest TAILL, highest LEFT-TAILL) go to the 1:1 leftover
    slots where the slot bias is exact.
    """
    p = np.arange(NS)
    blk, rem = p // 4096, p % 4096
    u4, pp2 = rem // 256, rem % 256
    pp3, e = pp2 % FB, pp2 // FB
    jmid = TAILL + 4096 * blk + 32 * pp3 + u4 + 16 * e
    t = p - NMID
    jtail = np.where(t < TAILL, t, NMID + t)
    return np.where(p < NMID, jmid, jtail)


_P2J = _pos_to_sorted()


def _slot_preimage():
    """slot -> up to 32 within-core SORTED RANKS (consecutive)."""
    pre = np.full((NSLOT, 32), -1, dtype=np.int64)
    s = np.arange(NSLOT)
    blk, pp3 = s // FB, s % FB
    full = blk < NPAIR
    pre[full] = (TAILL + 4096 * blk[full] + 32 * pp3[full])[:, None] \
        + np.arange(32)
    t = s[~full] - NPAIR * FB
    pre[~full, 0] = np.where(t < TAILL, t, NMID + t)
    return pre


_PRE = _slot_preimage()


def _build_program():
    import concourse.tile as tile
    from concourse import bacc, mybir

    F16 = mybir.dt.float16
    F32 = mybir.dt.float32
    U16 = mybir.dt.uint16

    nc = bacc.Bacc("TRN2", target_bir_lowering=False, debug=False,
                   num_devices=N_CORES)
    xT = nc.dram_tensor("xT", [D, T], F16, kind="ExternalInput").ap()
    xtrT = nc.dram_tensor("xtrT", [D, NS], F16, kind="ExternalInput").ap()
    sbias = nc.dram_tensor("sbias", [1, NSLOT], F16, kind="ExternalInput").ap()
    ones = nc.dram_tensor("ones", [1, D], F16, kind="ExternalInput").ap()
    out_vals = nc.dram_tensor("vals", [128, NG * NKEEP], F16,
                              kind="ExternalOutput").ap()
    out_idx = nc.dram_tensor("idx", [128, NG * NKEEP], U16,
                             kind="ExternalOutput").ap()

    mx = mybir.AluOpType.max
    ad = mybir.AluOpType.add

    def halves(ap_2d, width):
        """[128, 3*2w] -> two [128, 3, w] 3D APs (first/second halves)."""
        v = ap_2d.rearrange("p (a b) -> p a b", b=2 * width)
        return v[:, :, :width], v[:, :, width:]

    with tile.TileContext(nc) as tc:
        with ExitStack() as ctx:
            consts = ctx.enter_context(tc.tile_pool(name="consts", bufs=1))
            xT_sb = consts.tile([D, T], F16, name="xT_sb", tag="xT")
            nc.sync.dma_start(xT_sb[:], xT[:])
            ones_sb = consts.tile([1, D], F16, name="ones_sb", tag="ones")
            nc.sync.dma_start(ones_sb[:], ones[:])
            sb_row = consts.tile([1, NSLOT], F16, name="sb_row", tag="sbr")
            nc.sync.dma_start(sb_row[:], sbias[:])
            xtr_sb = consts.tile([D, NS], F16, name="xtr_sb", tag="xtr")
            # split the big upload so compute can start early
            for lo, hi in ((0, 2048), (2048, 4096), (4096, 8192),
                           (8192, NS)):
                nc.sync.dma_start(xtr_sb[:, lo:hi], xtrT[:, lo:hi])
            v8_all = consts.tile([128, NG * NKEEP], F16, name="v8a", tag="v8a")
            i8_all = consts.tile([128, NG * NKEEP], U16, name="i8a", tag="i8a")
            bias_bc = consts.tile([128, NSLOT], F16, name="bias_bc", tag="bb")

            psum = ctx.enter_context(tc.tile_pool(name="ps", bufs=2,
                                                  space="PSUM"))
            cpool = ctx.enter_context(tc.tile_pool(name="cp", bufs=6))
            mpool = ctx.enter_context(tc.tile_pool(name="mp", bufs=2))
            rpool = ctx.enter_context(tc.tile_pool(name="rp", bufs=2))
            vpool = ctx.enter_context(tc.tile_pool(name="vp", bufs=3))

            # broadcast the slot-bias row across partitions: ones^T @ sbias
            Pb = psum.tile([128, PTW], F32, name="P")
            nc.tensor.matmul(Pb[:, :BANK], ones_sb[:1, :], sb_row[:1, :BANK],
                             start=True, stop=True)
            nc.tensor.matmul(Pb[:, BANK:NSLOT], ones_sb[:1, :],
                             sb_row[:1, BANK:], start=True, stop=True)
            nc.scalar.copy(bias_bc[:], Pb[:, :NSLOT])

            for g in range(NG):
                lhs = xT_sb[:, g * 128:(g + 1) * 128]
                M = mpool.tile([128, NPAIR * PTW], F16, name="M")
                V2 = vpool.tile([128, NSLOT], F16, name="V2")
                prev = None
                for t in range(2 * NPAIR + 1):
                    P = psum.tile([128, PTW], F32, name="P")
                    off = t * PTW
                    width = PTW if t < 2 * NPAIR else LEFT
                    for lo in range(0, width, BANK):
                        hi = min(lo + BANK, width)
                        nc.tensor.matmul(P[:, lo:hi], lhs,
                                         xtr_sb[:, off + lo:off + hi],
                                         start=True, stop=True)
                    if t == 2 * NPAIR:       # leftover -> 1:1 slots
                        nc.scalar.copy(V2[:, NPAIR * FB:], P[:, :LEFT])
                        break
                    if t % 2 == 0:
                        if t // 2 >= NPAIR - CONVERT:
                            c = cpool.tile([128, PTW], F16, name="c")
                            nc.scalar.copy(c[:], P[:])
                            prev = ("sbuf", c)
                        else:
                            prev = ("psum", P)
                    else:
                        j = t // 2
                        mdst = M[:, j * PTW:(j + 1) * PTW]
                        kind, a = prev
                        if kind == "psum":
                            c = cpool.tile([128, PTW], F16, name="c")
                            nc.scalar.copy(c[:], P[:])
                            nc.vector.tensor_tensor(mdst, a[:], c[:], mx)
                        else:
                            nc.vector.tensor_tensor(mdst, P[:], a[:], mx)
                # fp16 fold tree, one multi-block op per level
                R = rpool.tile([128, NPAIR * 1024], F16, name="R")
                i0, i1 = halves(M[:], 1024)
                nc.vector.tensor_tensor(
                    R[:].rearrange("p (a b) -> p a b", b=1024), i0, i1, mx)
                R2 = rpool.tile([128, NPAIR * 512], F16, name="R2")
                i0, i1 = halves(R[:], 512)
                nc.vector.tensor_tensor(
                    R2[:].rearrange("p (a b) -> p a b", b=512), i0, i1, mx)
                V = vpool.tile([128, NPAIR * 256], F16, name="V")
                i0, i1 = halves(R2[:], 256)
                nc.vector.tensor_tensor(
                    V[:].rearrange("p (a b) -> p a b", b=256), i0, i1, mx)
                i0, i1 = halves(V[:], FB)
                nc.vector.tensor_tensor(
                    V2[:, :NPAIR * FB].rearrange("p (a b) -> p a b", b=FB),
                    i0, i1, mx)
                Ub = vpool.tile([128, NSLOT], F16, name="Ub")
                nc.vector.tensor_tensor(Ub[:], V2[:], bias_bc[:], ad)
                v8 = v8_all[:, g * NKEEP:(g + 1) * NKEEP]
                nc.vector.max(v8, Ub[:])
                nc.vector.max_index(i8_all[:, g * NKEEP:(g + 1) * NKEEP],
                                    v8, Ub[:])
            nc.sync.dma_start(out_vals[:], v8_all[:])
            nc.sync.dma_start(out_idx[:], i8_all[:])
    nc.compile()
    return nc


def _get_program():
    if "nc" not in _CACHE:
        _CACHE["nc"] = _build_program()
    return _CACHE["nc"]


def _prep(X, X_train):
    """Sort by norm, deal round-robin, interleave blocks; build inputs."""
    xT = np.ascontiguousarray(X.T.astype(np.float16))
    ones = np.ones((1, D), dtype=np.float16)
    q = np.einsum("nd,nd->n", X_train, X_train, dtype=np.float64)
    order = np.argsort(q, kind="stable")           # global sorted ranks
    in_maps, sorted_ids = [], []
    for c in range(N_CORES):
        Oc = order[c::N_CORES]                     # within-core sorted ids
        pos_ids = Oc[_P2J]                         # device position -> id
        xtrT = np.ascontiguousarray(X_train[pos_ids].T.astype(np.float16))
        qj = q[Oc]                                 # by sorted rank j
        sb = np.empty(NSLOT, dtype=np.float64)
        full = qj[TAILL:TAILL + NMID].reshape(NPAIR, FB, 32)
        sb[:NPAIR * FB] = -0.5 * full.max(axis=2).reshape(-1)
        tl = np.arange(LEFT)
        sb[NPAIR * FB:] = -0.5 * qj[np.where(tl < TAILL, tl, NMID + tl)]
        in_maps.append({"xT": xT, "xtrT": xtrT, "ones": ones,
                        "sbias": sb.astype(np.float16)[None, :]})
        sorted_ids.append(Oc)
    return in_maps, sorted_ids


def _prep_in_maps(X, X_train):
    return _prep(X, X_train)[0]


def _merge_and_vote(results, sorted_ids, X, X_train, y_train, k):
    all_vals = np.empty((T, N_CORES * NKEEP), dtype=np.float32)
    all_gid = np.empty((T, N_CORES * NKEEP, 32), dtype=np.int64)
    all_ok = np.empty((T, N_CORES * NKEEP, 32), dtype=bool)
    for c in range(N_CORES):
        v = results[c]["vals"].astype(np.float32)              # [128, 64]
        s = results[c]["idx"].astype(np.int64)
        v = v.reshape(128, NG, NKEEP).transpose(1, 0, 2).reshape(T, NKEEP)
        s = s.reshape(128, NG, NKEEP).transpose(1, 0, 2).reshape(T, NKEEP)
        pre = _PRE[s]                                          # [T, NKEEP, 32]
        ok = pre >= 0
        gid = sorted_ids[c][np.where(ok, pre, 0)]
        sl = slice(c * NKEEP, (c + 1) * NKEEP)
        all_vals[:, sl] = v
        all_gid[:, sl] = gid
        all_ok[:, sl] = ok

    def rescore(sets):
        cands = np.take_along_axis(all_gid, sets[:, :, None], axis=1)
        valid = np.take_along_axis(all_ok, sets[:, :, None], axis=1)
        flat = np.where(valid, cands, 0).reshape(T, -1)
        vecs = X_train[flat].astype(np.float64)                # [T, M, D]
        s = np.matmul(vecs, X.astype(np.float64)[:, :, None])[:, :, 0]
        s -= 0.5 * np.einsum("tmd,tmd->tm", vecs, vecs)
        s[~valid.reshape(T, -1)] = -np.inf
        return flat, s

    srt = np.argsort(-all_vals, axis=1)                        # [T, 64]
    flat1, s1 = rescore(srt[:, :8])
    kth = -np.sort(-s1, axis=1)[:, k - 1]                      # provisional
    vals_s = np.take_along_axis(all_vals, srt, axis=1)
    need = vals_s[:, 8:] >= (kth[:, None] - 0.6)               # upper bounds
    nmore = int(need.sum(axis=1).max())
    if nmore > 0:
        flat2, s2 = rescore(srt[:, 8:8 + nmore])
        s2[np.repeat(~need[:, :nmore], 32, axis=1)] = -np.inf
        flat1 = np.concatenate([flat1, flat2], axis=1)
        s1 = np.concatenate([s1, s2], axis=1)
    order = np.lexsort((flat1, -s1), axis=1)[:, :k]
    idx_k = np.take_along_axis(flat1, order, axis=1)
    labels = y_train[idx_k]                                    # [T, k]
    counts = (labels[:, :, None] == np.arange(NUM_CLASSES)).sum(axis=1)
    return np.argmax(counts, axis=1).astype(np.float32)


def kernel(X, X_train, y_train, k):
    from concourse.bass_utils import run_bass_kernel_spmd

    X = np.asarray(X, dtype=np.float32)
    X_train = np.asarray(X_train, dtype=np.float32)
    y_train = np.asarray(y_train)
    k = int(k)
    assert X.shape == (T, D) and X_train.shape == (N_TRAIN, D)
    assert 1 <= k <= 8

    nc = _get_program()
    in_maps, sorted_ids = _prep(X, X_train)
    res = run_bass_kernel_spmd(nc, in_maps, core_ids=list(range(N_CORES)))
    return _merge_and_vote(res.results, sorted_ids, X, X_train, y_train, k)
